# revision 11
# baseline (speedup 1.0000x reference)
"""Trainium2 Bass kernel for: Conv3d(3->16, k=3x3x3, VALID) + bias -> min over
depth -> softmax over channels.

Input  x: (16, 3, 32, 128, 128) f32   [N, C_in, D, H, W]
Weight w: (16, 3, 3, 3, 3) f32        [C_out, C_in, kD, kH, kW]
Bias   b: (16,) f32
Output  : (16, 16, 126, 126) f32      [N, C_out, H_out, W_out]

Data-parallel over batch: 2 batches per core x 8 cores. Per core:

  - x stored per (batch, h-half) as one [128, 8768] bf16 tile: strip r
    (partition quadrant 32r) holds 30 rows = (10 input depths 8r..8r+9) x
    (ci 3); free dim = local (h, w) flattened (66 or 64 h-rows + pad).
  - Conv as 4 row-packed matmuls per (chunk, khw): tile r = [K<=30, M=128,
    N=512] at tile_position (32r, 0); M = 8 local douts x 16 co; 9
    accumulating MMs over (kh,kw) with free-dim-shifted rhs (stride-1 conv
    == same spatial layout + offset koff). Weight col block[(dl,ci),
    (dll,co)] = w[co,ci,dl-dll,kh,kw]. PSUM supertile [128, 4*512]:
    bank r = strip r's 8 douts. PSUM holds ONLY conv tiles (2-deep
    ping-pong, all 8 banks) -- the softmax tail never allocates PSUM, so
    the ring phase is never disturbed and the PE never waits on the long
    cross-engine softmax chain.
  - Strip 3 douts 30,31 don't exist: their weight cols are 0 except a
    BIG=32768 entry at khw=0 against a constant-1.0 rhs row, so those psum
    lanes hold +32768 and never win the depth-min.
  - Depth-min: ACT (scalar) copies the 4 psum banks to SBUF bf16, then DVE
    folds them with two contiguous bf16 tensor_tensor mins (2 elem/cyc),
    then a 128->64->32->16 partition tree-min (DMA shift + TT min; level 1
    on GpSimd, levels 2-3 on DVE) collapses the 8 dout_local groups.
  - Softmax over co per 8-chunk group, with no PE matmuls: one DMA
    regroups the [16 co, 8 chunks * 512] mins into [128 = 8co+j, 512];
    ACT exp (bias fused; min(y)+b == min(y+b)); co-sums via a 4-level
    partition fold (DMA shift + GpSimd f32 add over stride-8 co groups);
    DVE reciprocal into rb[0:8]; rb broadcast up to 128 partitions with 4
    doubling DMAs; DVE multiply; one DMA out per group (y DRAM padded to
    [NB, 16, 128, 128], sliced on host).
"""

import os
import sys

sys.path.insert(0, "/opt/trn_rl_repo")

import numpy as np
import ml_dtypes

import concourse.bass as bass
import concourse.bacc as bacc
import concourse.tile as tile
import concourse.mybir as mybir
import concourse.bass_isa as bass_isa
from concourse import bass_utils

F32 = mybir.dt.float32
BF16 = mybir.dt.bfloat16

N_CORES = 8
NB = 2           # batches per core
CI = 3
D = 32
H = 128
W = 128
CO = 16
CHUNK = 512
HOUT = 126
WOUT = 126
PAD = 320
QF = 66 * W + PAD  # quad tile free size (worst case hh=0)
BIG = 32768.0

_COMPILED = {}


def _strip_depths(r):
    return 10 if r < 3 else 8


def _kr(r, khw):
    if r < 3:
        return 30
    return 25 if khw == 0 else 24


def _build_weight_blocks(conv_weight):
    """[128, 9*128]: strip r rows 32r+(3*dl+ci); col khw*128 + dll*16 + co
    = w[co, ci, dl-dll, kh, kw] (0 outside kd range / dout>=30). Row 120
    (strip 3 local 24) carries BIG at khw=0 for the dout 30/31 lanes."""
    wb = np.zeros((128, 9 * 128), dtype=np.float32)
    for r in range(4):
        for dl in range(_strip_depths(r)):
            for ci in range(CI):
                row = 32 * r + 3 * dl + ci
                for khw in range(9):
                    kh, kw = khw // 3, khw % 3
                    for dll in range(8):
                        kd = dl - dll
                        if 8 * r + dll < 30 and 0 <= kd <= 2:
                            wb[row, khw * 128 + dll * 16:
                               khw * 128 + dll * 16 + 16] = \
                                conv_weight[:, ci, kd, kh, kw]
    for dll in (6, 7):
        wb[120, dll * 16:dll * 16 + 16] = BIG
    return wb.astype(ml_dtypes.bfloat16)


def _build_bias128(conv_bias):
    """[128, 1]: partition 8*co + j -> bias[co]."""
    b = np.zeros((128, 1), dtype=np.float32)
    for p in range(128):
        b[p, 0] = conv_bias[p // 8]
    return b


def _emit_kernel(tc):
    nc = tc.nc
    x_ap = nc.dram_tensor("x", [NB, D, CI, H, W], BF16,
                          kind="ExternalInput").ap()
    w_ap = nc.dram_tensor("w", [128, 9 * 128], BF16,
                          kind="ExternalInput").ap()
    bias_ap = nc.dram_tensor("bias", [128, 1], F32, kind="ExternalInput").ap()
    # h/w-padded output; host slices [:, :, :126, :126]
    y_ap = nc.dram_tensor("y", [NB, CO, 128, 128], F32,
                          kind="ExternalOutput").ap()

    from contextlib import ExitStack

    with ExitStack() as ctx:
        const_pool = ctx.enter_context(tc.tile_pool(name="const", bufs=1))
        in_pool = ctx.enter_context(tc.tile_pool(name="in", bufs=2))
        m4w_pool = ctx.enter_context(tc.tile_pool(name="m4w", bufs=2))
        ev_pool = ctx.enter_context(tc.tile_pool(name="ev", bufs=3))
        sm_pool = ctx.enter_context(tc.tile_pool(name="sm", bufs=2))
        psum_pool = ctx.enter_context(tc.tile_pool(name="ps", bufs=2,
                                                   space="PSUM"))

        w_sb = const_pool.tile([128, 9 * 128], BF16, tag="w")
        nc.sync.dma_start(w_sb[:, :], w_ap[:, :])
        bias_sb = const_pool.tile([128, 1], F32, tag="bias")
        nc.sync.dma_start(bias_sb[:, :], bias_ap[:, :])

        def emit_tail_a(t):
            m4w_t, n_t, h0_t, q_t, st = t
            # fold the 8 dout_local groups (partition tree-min); engine TTs
            # need equal SBUF partition bases, so shift the upper half down
            # via SBUF-to-SBUF DMA at each level. Level 1 runs on GpSimd to
            # keep DVE under the PE's chunk rate.
            sh = sm_pool.tile([64, 8 * CHUNK], BF16, tag="sh")
            nc.sync.dma_start(sh[0:64, :], m4w_t[64:128, :])
            nc.vector.tensor_tensor(
                out=m4w_t[0:64, :], in0=m4w_t[0:64, :], in1=sh[0:64, :],
                op=mybir.AluOpType.min)
            nc.sync.dma_start(sh[0:32, :], m4w_t[32:64, :])
            nc.vector.tensor_tensor(
                out=m4w_t[0:32, :], in0=m4w_t[0:32, :], in1=sh[0:32, :],
                op=mybir.AluOpType.min)
            nc.sync.dma_start(sh[0:16, :], m4w_t[16:32, :])
            nc.vector.tensor_tensor(
                out=m4w_t[0:16, :], in0=m4w_t[0:16, :], in1=sh[0:16, :],
                op=mybir.AluOpType.min)
            # regroup mins to the softmax layout in one DMA:
            # out partition p = 8*co + j  <-  in walk (co, j, s)
            coll = sm_pool.tile([128, CHUNK], BF16, tag="coll")
            nc.sync.dma_start(
                coll[:, :],
                m4w_t[0:16, :].rearrange("co (j s) -> co j s", j=8))
            expt = sm_pool.tile([128, CHUNK], BF16, tag="exp")
            nc.scalar.activation(
                expt[:, :], coll[:, :],
                mybir.ActivationFunctionType.Exp,
                bias=bias_sb[:, :], scale=1.0)
            st["expt"] = expt
            es = sm_pool.tile([64, CHUNK], BF16, tag="es")
            esh = sm_pool.tile([64, CHUNK], BF16, tag="esh")
            es8 = sm_pool.tile([8, CHUNK], F32, tag="es8")
            st["es"] = es
            st["esh"] = esh
            st["es8"] = es8

        def emit_esum_fold(t, level):
            # co-sum of expt: p = 8co+j, so summing stride-8 partition
            # groups = folding the top half down, 4 times (co 16 -> 1).
            # bf16 until the last level, which lands in f32 for reciprocal.
            m4w_t, n_t, h0_t, q_t, st = t
            expt, es, esh = st["expt"], st["es"], st["esh"]
            ctx_lp = nc.allow_low_precision(
                "softmax denom: bf16 partial sums are within the 2e-2 "
                "output tolerance")
            ctx_lp.__enter__()
            if level == 0:
                nc.sync.dma_start(esh[0:64, :], expt[64:128, :])
                nc.vector.tensor_tensor(
                    out=es[0:64, :], in0=expt[0:64, :], in1=esh[0:64, :],
                    op=mybir.AluOpType.add)
            elif level < 3:
                w_ = 64 >> level
                nc.sync.dma_start(esh[0:w_, :], es[w_:2 * w_, :])
                nc.vector.tensor_tensor(
                    out=es[0:w_, :], in0=es[0:w_, :], in1=esh[0:w_, :],
                    op=mybir.AluOpType.add)
            else:
                nc.sync.dma_start(esh[0:8, :], es[8:16, :])
                nc.vector.tensor_tensor(
                    out=st["es8"][0:8, :], in0=es[0:8, :], in1=esh[0:8, :],
                    op=mybir.AluOpType.add)
            ctx_lp.__exit__(None, None, None)

        def emit_tail_b(t):
            m4w_t, n_t, h0_t, q_t, st = t
            rb = sm_pool.tile([128, CHUNK], F32, tag="rb")
            nc.vector.reciprocal_approx_fast(rb[0:8, :], st["es8"][0:8, :])
            st["rb"] = rb

        def emit_tail_c(t):
            m4w_t, n_t, h0_t, q_t, st = t
            rb = st["rb"]
            for w_ in (8, 16, 32, 64):
                nc.sync.dma_start(rb[w_:2 * w_, :], rb[0:w_, :])
            soft = sm_pool.tile([128, CHUNK], F32, tag="soft")
            nc.vector.tensor_tensor(
                out=soft[:, :], in0=rb[:, :], in1=st["expt"][:, :],
                op=mybir.AluOpType.mult)
            # one DMA: dst y[n, co, h0+32q .. +32, 0:128] (padded DRAM)
            # walk (co, h=(j,i), w) == src walk (p=8co+j, s=(i,w))
            hc = h0_t + 32 * q_t
            dst = y_ap[n_t, :, hc:hc + 32, :].rearrange("co h w -> co (h w)")
            nc.sync.dma_start(dst, soft[:, :])

        pending = None
        cur = {}
        items = [(n, hh) for n in range(NB) for hh in range(2)]
        quads = {}

        def prep_quad(k):
            n_k, hh_k = items[k]
            h0_k = 64 * hh_k
            hrows = 66 if hh_k == 0 else 64
            quad_k = in_pool.tile([128, QF], BF16, tag="quad")
            nc.gpsimd.memset(quad_k[:, hrows * W:QF], 0.0)
            # rows 96..127 <- 1.0; strip-3 DMA overwrites 96..119, leaving
            # the constant-1.0 row 120 for the BIG sentinel matmul column.
            nc.gpsimd.memset(quad_k[96:128, 0:hrows * W], 1.0)
            for r in range(4):
                nd = _strip_depths(r)
                srcr = x_ap[n_k, 8 * r:8 * r + nd, :,
                            h0_k:h0_k + hrows, :].rearrange(
                    "d c h w -> (d c) (h w)")
                nc.sync.dma_start(
                    quad_k[32 * r:32 * r + 3 * nd, 0:hrows * W], srcr)
            quads[k] = quad_k

        prep_quad(0)
        for k, (n, hh) in enumerate(items):
            h0 = 64 * hh
            quad = quads.pop(k)
            if True:
                for q in range(2):
                    m4w = m4w_pool.tile([128, 8 * CHUNK], BF16, tag="m4w")
                    for j in range(8):
                        m = 8 * q + j
                        s0 = CHUNK * m
                        ps = psum_pool.tile([128, 4 * CHUNK], F32, tag="big")
                        for khw in range(9):
                            kh, kw = khw // 3, khw % 3
                            koff = kh * W + kw
                            for r in range(4):
                                kr = _kr(r, khw)
                                nc.tensor.matmul(
                                    ps[:, r * CHUNK:(r + 1) * CHUNK],
                                    lhsT=w_sb[32 * r:32 * r + kr,
                                              khw * 128:(khw + 1) * 128],
                                    rhs=quad[32 * r:32 * r + kr,
                                             s0 + koff:s0 + koff + CHUNK],
                                    start=(khw == 0),
                                    stop=(khw == 8),
                                    tile_position=(32 * r, 0),
                                    skip_group_check=True,
                                )
                        # evacuate psum on ACT (f32 -> bf16), then fold the
                        # 4 banks with two contiguous bf16 mins on DVE,
                        # merged across chunk pairs to amortize DVE op
                        # overhead (the 151-cycle fixed cost per op).
                        if j % 2 == 0:
                            c4 = ev_pool.tile([128, 8 * CHUNK], BF16,
                                              tag="c4")
                            cur["c4"] = c4
                        c4 = cur["c4"]
                        half = (j % 2) * 4 * CHUNK
                        nc.scalar.copy(c4[:, half:half + 4 * CHUNK],
                                       ps[:, :])
                        if j % 2 == 1:
                            h2 = ev_pool.tile([128, 4 * CHUNK], BF16,
                                              tag="h2")
                            c4v = c4.rearrange("p (c x) -> p c x", c=2)
                            nc.vector.tensor_tensor(
                                out=h2.rearrange("p (c x) -> p c x", c=2),
                                in0=c4v[:, :, 0:2 * CHUNK],
                                in1=c4v[:, :, 2 * CHUNK:4 * CHUNK],
                                op=mybir.AluOpType.min)
                            h2v = h2.rearrange("p (c x) -> p c x", c=2)
                            nc.vector.tensor_tensor(
                                out=m4w[:, (j - 1) * CHUNK:(j + 1) * CHUNK]
                                .rearrange("p (c x) -> p c x", c=2),
                                in0=h2v[:, :, 0:CHUNK],
                                in1=h2v[:, :, CHUNK:2 * CHUNK],
                                op=mybir.AluOpType.min)
                        # software-pipelined: previous group's tail ops
                        # land between this group's chunks so every engine
                        # FIFO sees them with (nearly) ready inputs.
                        if pending is not None:
                            if j == 0:
                                emit_tail_a(pending)
                            elif 2 <= j <= 5:
                                emit_esum_fold(pending, j - 2)
                            elif j == 6:
                                emit_tail_b(pending)
                            elif j == 7:
                                emit_tail_c(pending)
                                pending = None
                        if q == 0 and j == 1 and k + 1 < len(items):
                            prep_quad(k + 1)
                    pending = (m4w, n, h0, q, {})
        emit_tail_a(pending)
        for lv in range(4):
            emit_esum_fold(pending, lv)
        emit_tail_b(pending)
        emit_tail_c(pending)


def _compile():
    if "nc" in _COMPILED:
        return _COMPILED["nc"]
    nc = bacc.Bacc("TRN2", target_bir_lowering=False, debug=False,
                   num_devices=N_CORES)
    with tile.TileContext(nc) as tc:
        _emit_kernel(tc)
    nc.compile()
    _COMPILED["nc"] = nc
    return nc


def kernel(x, conv_weight, conv_bias):
    x = np.asarray(x, dtype=np.float32)
    conv_weight = np.asarray(conv_weight, dtype=np.float32)
    conv_bias = np.asarray(conv_bias, dtype=np.float32)

    xp = np.ascontiguousarray(
        x.transpose(0, 2, 1, 3, 4)).astype(ml_dtypes.bfloat16)  # [N,D,C,H,W]
    w_sb = _build_weight_blocks(conv_weight)
    bias_sb = _build_bias128(conv_bias)

    nc = _compile()
    in_maps = []
    for i in range(N_CORES):
        in_maps.append({
            "x": np.ascontiguousarray(xp[NB * i:NB * (i + 1)]),
            "w": w_sb,
            "bias": bias_sb,
        })
    res = bass_utils.run_bass_kernel_spmd(
        nc, in_maps, core_ids=list(range(N_CORES)),
        trace=bool(int(os.environ.get("KERNEL_TRACE", "0"))),
    )
    _COMPILED["last_results"] = res
    out = np.concatenate(
        [res.results[i]["y"][:, :, :HOUT, :WOUT] for i in range(N_CORES)],
        axis=0)
    return out


if __name__ == "__main__":
    _compile()
    print("build OK")
